# revision 30
# baseline (speedup 1.0000x reference)
"""Trainium2 Bass kernel for dynamic low-pass filter decomposition.

Module: global-avg-pool -> 1x1 conv -> BN -> softmax over 3x3 taps gives a
per-(sample, group) 3x3 kernel; applied as a reflect-padded depthwise conv
over x; returns (low, x - low).

Sharding: data-parallel over batch n=8 across 8 NeuronCores (1 sample/core).

Per-core layout: partition p = c*2 + h (h = row-half of the image, c =
channel).  Each partition holds 98 rows x 192 cols of its (channel, half)
with one halo row above/below (reflection resolved at DMA time by source row
choice) plus a 1-element front/back pad so tap-shifted views stay in bounds.

Engine split: the TensorEngine accumulates 8 of the 9 taps as diagonal fp32r
matmuls per 512-col PSUM chunk; ScalarE copies the partial out of PSUM;
VectorE adds the 9th (center) tap in-place in SBUF, fixes the reflect
columns at w=0/191 (6 merged ops) and computes high = x - low.  Consts load
on the GpSimd DMA queue so they never queue behind input super-tiles;
warm-up matmuls keep the PE p-state ramped through the input phase; the
1x1-conv/BN/softmax weight chain runs from exact per-chunk partial sums
(fp32) with BN folded into the conv weights on the host.
"""
import sys
import os

sys.path.insert(0, "/opt/trn_rl_repo")

import numpy as np
from contextlib import ExitStack

import concourse.bass as bass
import concourse.tile as tile
from concourse import bacc, mybir
from concourse.bass_utils import run_bass_kernel_spmd

dt = mybir.dt
f32 = dt.float32

KS = 3
GROUP = 8
IC = 64
BN_EPS = 1e-5
N = 8
H = W = 192
RH = 96                 # rows per half-image
NB = 98 * W             # buffer elems per partition (98 rows of 192)
PAD = 1                 # front pad elems (also 1 at the back)
CH = 512                # matmul chunk (one PSUM bank)
# input load chunks (offset, size) in image-region elems, alternating
# between the sync and scalar DMA queues with exactly balanced bytes per
# queue; sizes shrink toward the end so the last partial-sum lands early
LOAD_CHUNKS = [(0, 3072), (3072, 3072), (6144, 3072), (9216, 3072),
               (12288, 2304), (14592, 2304), (16896, 768), (17664, 768)]
SYNC_LOADS = (0, 2, 4, 6)      # chunk idx -> sync queue, rest on scalar
WARM_PER_CHUNK = (2, 2, 2, 2, 2, 1, 1, 1)
WARM_BRIDGE = 4
WARM_CHAIN = 3                 # warm matmuls slotted between chain matmuls
ST_ROWS = [16, 16, 16, 16, 16, 8, 4, 4]   # compute super-tile heights
PE_ALL9 = (6, 7)               # drain STs: PE does all 9 taps (short V tail)
PE_TAPS = (0, 1, 2, 3, 5, 6, 7, 8)
V_TAP = 4                      # center tap, added in-place in SBUF


def _build_program():
    """Trace the SPMD Bass program (same for every core)."""
    nc = bacc.Bacc("TRN2", target_bir_lowering=False, debug=False,
                   num_devices=N)

    x_d = nc.dram_tensor("x", [64, H, W], dt.float32r, kind="ExternalInput")
    at_d = nc.dram_tensor("at128", [128, 72], f32, kind="ExternalInput")
    b_d = nc.dram_tensor("b72", [72, 1], f32, kind="ExternalInput")
    r9_d = nc.dram_tensor("r9", [72, 9], f32, kind="ExternalInput")
    g_d = nc.dram_tensor("g728", [72, 8], f32, kind="ExternalInput")
    h_d = nc.dram_tensor("h8128", [8, 128], f32, kind="ExternalInput")
    eye_d = nc.dram_tensor("eye", [128, 128], f32, kind="ExternalInput")
    eyer_d = nc.dram_tensor("eyer", [128, 128], dt.float32r,
                            kind="ExternalInput")
    low_d = nc.dram_tensor("low", [64, H, W], f32, kind="ExternalOutput")
    high_d = nc.dram_tensor("high", [64, H, W], f32, kind="ExternalOutput")

    xt_dram = x_d.ap()

    def dram_flat(tensor, base, inner):
        """Flat (128, inner) AP over DRAM: partition p = c*2 + h covers
        x.flat[p*18432 + base : ... + inner].  Flat leading-dim-128 APs get
        the full 16-engine DMA spray (~430 GB/s); (h,c)-interleaved ones
        only engage 2 engines (~53 GB/s measured)."""
        return bass.AP(tensor, base, [[RH * W, 128], [1, inner]])

    with tile.TileContext(nc) as tc, ExitStack() as ctx:
        cpool = ctx.enter_context(tc.tile_pool(name="static", bufs=1))
        xpool = wpool = cpool
        spool = ctx.enter_context(tc.tile_pool(name="stage", bufs=3))

        # ---- x chunk loads FIRST (queue FIFO position = landing time) on
        # sync+scalar queues; halos behind them on sync; consts go on the
        # GpSimd queue so they land early without delaying x ----
        xt = xpool.tile([128, PAD + NB + 1], dt.float32r)
        # the 1-elem front/back pads are read (never used) by tap-shifted
        # edge views; zero them so they hold no junk/NaN
        nc.vector.memset(xt[:, 0:PAD].bitcast(f32), 0.0)
        nc.vector.memset(xt[:, PAD + NB:PAD + NB + 1].bitcast(f32), 0.0)
        for i, (off, sz) in enumerate(LOAD_CHUNKS):
            eng = nc.sync if i in SYNC_LOADS else nc.scalar
            eng.dma_start(xt[:, PAD + W + off:PAD + W + off + sz],
                          dram_flat(xt_dram.tensor, off, sz))
        # halo row 97 <- image rows {96, 190 (reflect)}[h]; only needed by
        # the last super-tile, so it queues behind sync's chunks
        nc.sync.dma_start(xt[:, PAD + 97 * W:PAD + 98 * W],
                          bass.AP(xt_dram.tensor, 96 * W,
                                  [[H * W, 64], [94 * W, 2], [1, W]]))

        at_s = cpool.tile([128, 72], f32)
        b_s = cpool.tile([72, 1], f32)
        r9_s = cpool.tile([72, 9], f32)
        g_s = cpool.tile([72, 8], f32)
        h_s = cpool.tile([8, 128], f32)
        eye_s = cpool.tile([128, 128], f32)
        eyer_s = cpool.tile([128, 128], dt.float32r)
        for t, d in ((eyer_s, eyer_d), (at_s, at_d), (b_s, b_d),
                     (r9_s, r9_d), (g_s, g_d), (h_s, h_d), (eye_s, eye_d)):
            nc.gpsimd.dma_start(t[:], d.ap())
        # halo row 0 <- image rows {1 (reflect), 95}[h] (8th gp issue: the gp
        # queue has 8 descriptor slots)
        nc.gpsimd.dma_start(xt[:, PAD:PAD + W],
                            bass.AP(xt_dram.tensor, W,
                                    [[H * W, 64], [94 * W, 2], [1, W]]))

        # ---- partial sums per chunk (VectorE reduce / ScalarE accumulate)
        # into one [128, 7] tile; engines split so both overlap the DMAs ----
        pt = wpool.tile([128, 8], f32)
        rscratch = wpool.tile([128, 3072], f32)
        edummy = wpool.tile([72, 1], f32)
        vcol = scol = 0
        for i, (off, sz) in enumerate(LOAD_CHUNKS):
            src = xt[:, PAD + W + off:PAD + W + off + sz].bitcast(f32)
            if i in SYNC_LOADS:
                nc.vector.tensor_reduce(pt[:, vcol:vcol + 1], src,
                                        axis=mybir.AxisListType.X,
                                        op=mybir.AluOpType.add)
                vcol += 1
            else:
                nc.scalar.activation(rscratch[:, 0:sz], src,
                                     mybir.ActivationFunctionType.Copy,
                                     accum_out=pt[:, 4 + scol:5 + scol])
                scol += 1
                if scol == 1:
                    # dummy Exp so the activation table holding Exp is
                    # loaded during the input phase, not on the chain's
                    # critical path
                    nc.scalar.activation(edummy[:], b_s[:],
                                         mybir.ActivationFunctionType.Exp)

        # ---- weight generation chain, sharing one PSUM pool with the PE
        # warm-up matmuls (fewer pools = fewer teardown drain rounds) ----
        sum128 = wpool.tile([128, 1], f32)
        w128 = wpool.tile([128, 9], f32)
        wc = wpool.tile([128, 3], f32)
        diag = [wpool.tile([128, 128], dt.float32r, name=f"diag{k}")
                for k in range(9)]
        wa = PAD + W

        with tc.tile_pool(name="wpsum", bufs=1,
                          space=bass.MemorySpace.PSUM) as wpsum:
            # PE warm-up: dummy matmuls chained to each chunk's landing keep
            # the p-state ramped through the otherwise PE-idle input phase
            wrm2 = wpsum.tile([128, CH], f32, tag="wrm2")
            for i, (off, sz) in enumerate(LOAD_CHUNKS):
                a = PAD + W + off
                for j in range(WARM_PER_CHUNK[i]):
                    nc.tensor.matmul(wrm2[:], eyer_s[:], xt[:, a:a + CH])
            a = PAD + W + LOAD_CHUNKS[-1][0]
            for j in range(WARM_BRIDGE):
                nc.tensor.matmul(wrm2[:], eyer_s[:], xt[:, a:a + CH])

            def warm_chain():
                for j in range(WARM_CHAIN):
                    nc.tensor.matmul(wrm2[:], eyer_s[:], xt[:, wa:wa + CH])

            nc.vector.tensor_reduce(sum128[:], pt[:, 0:8],
                                    axis=mybir.AxisListType.X,
                                    op=mybir.AluOpType.add)
            lf_p = wpsum.tile([72, 1], f32, tag="lf")
            nc.tensor.matmul(lf_p[:], at_s[:], sum128[:])
            warm_chain()
            e72 = wpool.tile([72, 1], f32)
            nc.scalar.activation(e72[:], lf_p[:],
                                 mybir.ActivationFunctionType.Exp,
                                 bias=b_s[:, 0:1], scale=1.0)
            rhsw = wpool.tile([72, 9], f32)
            nc.vector.tensor_scalar_mul(rhsw[:], r9_s[:], e72[:, 0:1])
            w89_p = wpsum.tile([8, 9], f32, tag="w89")
            nc.tensor.matmul(w89_p[:], g_s[:], rhsw[:])
            warm_chain()
            s8 = wpool.tile([8, 1], f32)
            nc.vector.tensor_reduce(s8[:], w89_p[:],
                                    axis=mybir.AxisListType.X,
                                    op=mybir.AluOpType.add)
            r8 = wpool.tile([8, 1], f32)
            nc.vector.reciprocal(r8[:], s8[:])
            w89s = wpool.tile([8, 9], f32)
            nc.vector.tensor_scalar_mul(w89s[:], w89_p[:], r8[:, 0:1])
            wbig_p = wpsum.tile([128, 9], f32, tag="wbig")
            nc.tensor.matmul(wbig_p[:], h_s[:], w89s[:])
            warm_chain()
            # w128 to SBUF; everything downstream (diags, V tap, edge fixes)
            # reads SBUF so this pool closes without gating the main loop
            nc.scalar.copy(w128[:], wbig_p[:])
        # diagonal weight matrices, in ST0's tap order so the PE never waits
        # on a later diag
        for k in PE_TAPS:
            nc.vector.tensor_scalar_mul(diag[k][:], eye_s[:],
                                        w128[:, k:k + 1])
        nc.vector.tensor_scalar_mul(diag[V_TAP][:], eye_s[:],
                                    w128[:, V_TAP:V_TAP + 1])
        # wc merges the dj=0/dj=2 weights hitting the mirror neighbour of a
        # reflected edge column
        nc.vector.tensor_tensor(wc[:], w128[:, 0:9:3], w128[:, 2:9:3],
                                op=mybir.AluOpType.add)

        # ---- main loop ----
        with tc.tile_pool(name="psum", bufs=8,
                          space=bass.MemorySpace.PSUM) as psum:
            r0 = 0
            for s, rows in enumerate(ST_ROWS):
                stw = rows * W
                base = PAD + W + r0 * W
                chunks = []
                o = 0
                while o < stw:
                    chunks.append((o, min(CH, stw - o)))
                    o += CH

                def tap_view(k, lo, sz):
                    di, dj = k // 3, k % 3
                    off = base + lo + (di - 1) * W + (dj - 1)
                    return xt[:, off:off + sz]

                acc = [psum.tile([128, csz], f32, tag="acc",
                                 name=f"acc{s}_{i}")
                       for i, (co, csz) in enumerate(chunks)]
                all9 = s in PE_ALL9
                pe_taps = tuple(range(9)) if all9 else PE_TAPS
                taps = pe_taps if s % 2 == 0 else pe_taps[::-1]
                for k in taps:
                    for i, (co, csz) in enumerate(chunks):
                        nc.tensor.matmul(acc[i][:], diag[k][:],
                                         tap_view(k, co, csz),
                                         start=(k == taps[0]),
                                         stop=(k == taps[-1]))
                low_st = spool.tile([128, stw], f32, tag="low",
                                    padded_shape=[128, 3072])
                # ScalarE drains PSUM; VectorE adds the center tap in-place
                # in SBUF right behind it, chunk by chunk (drain STs do all
                # 9 taps on the PE so the final V tail stays short)
                for i, (co, csz) in enumerate(chunks):
                    dst = low_st[:, co:co + csz]
                    nc.scalar.copy(dst, acc[i][:])
                    if not all9:
                        nc.vector.scalar_tensor_tensor(
                            dst, tap_view(V_TAP, co, csz).bitcast(f32),
                            w128[:, V_TAP:V_TAP + 1], dst,
                            op0=mybir.AluOpType.mult,
                            op1=mybir.AluOpType.add)
                # edge-column fixes (reflect at w=0 and w=191): per di, the
                # edge output is w[di,1]*x[.,edge] + (w[di,0]+w[di,2])*x[.,
                # mirror-neighbour]; both columns per op via strided views
                out_ap = low_st[:, 0:stw].rearrange(
                    "p (r w) -> p r w", w=W)[:, :, 0:W:W - 1]
                for di in range(3):
                    vb = PAD + (r0 + di) * W
                    va = xt[:, vb:vb + stw].bitcast(f32).rearrange(
                        "p (r w) -> p r w", w=W)[:, :, 0:W:W - 1]
                    vn = xt[:, vb + 1:vb + 1 + stw].bitcast(f32).rearrange(
                        "p (r w) -> p r w", w=W)[:, :, 0:190:189]
                    if di == 0:
                        nc.vector.tensor_scalar_mul(out_ap, va, w128[:, 1:2])
                    else:
                        nc.vector.scalar_tensor_tensor(
                            out_ap, va, w128[:, 3 * di + 1:3 * di + 2],
                            out_ap, op0=mybir.AluOpType.mult,
                            op1=mybir.AluOpType.add)
                    nc.vector.scalar_tensor_tensor(
                        out_ap, vn, wc[:, di:di + 1], out_ap,
                        op0=mybir.AluOpType.mult,
                        op1=mybir.AluOpType.add)
                high_st = spool.tile([128, stw], f32, tag="high",
                                     padded_shape=[128, 3072])
                nc.vector.tensor_tensor(high_st[:],
                                        xt[:, base:base + stw].bitcast(f32),
                                        low_st[:],
                                        op=mybir.AluOpType.subtract)
                nc.scalar.dma_start(
                    dram_flat(low_d.ap().tensor, r0 * W, stw), low_st[:])
                nc.sync.dma_start(
                    dram_flat(high_d.ap().tensor, r0 * W, stw), high_st[:])
                r0 += rows

    nc.compile()
    return nc


def _enable_ldw_opt():
    """walrus emits one LDWEIGHTS per matmul with --enable-ldw-opt=false
    (most are redundant reloads of the same diagonal).  Rewrite the flag on
    the compiler command line."""
    import concourse.bass_utils as BU
    if getattr(BU, "_ldw_patched", False):
        return
    orig = BU.run_command

    def patched(cmd, *a, **kw):
        cmd = [c.replace("--enable-ldw-opt=false", "--enable-ldw-opt=true")
               if isinstance(c, str) else c for c in cmd]
        return orig(cmd, *a, **kw)

    BU.run_command = patched
    BU._ldw_patched = True


_nc_cache = None


def _get_program():
    global _nc_cache
    if _nc_cache is None:
        _enable_ldw_opt()
        _nc_cache = _build_program()
    return _nc_cache


def _host_consts(conv_w, bn_gamma, bn_beta, bn_mean, bn_var):
    s_a = bn_gamma / np.sqrt(bn_var + BN_EPS)
    b72 = (bn_beta - bn_mean * s_a).astype(np.float32).reshape(72, 1)
    A = (conv_w * s_a[:, None]) / np.float32(H * W)
    p = np.arange(128)
    at128 = np.ascontiguousarray(A.T[p // 2]).astype(np.float32)  # (128, 72)
    oc = np.arange(72)
    r9 = (oc[:, None] % 9 == np.arange(9)[None, :]).astype(np.float32)
    g728 = (oc[:, None] // 9 == np.arange(8)[None, :]).astype(np.float32)
    h8128 = (np.arange(8)[:, None] == (p[None, :] // 16)).astype(np.float32)
    eye = np.eye(128, dtype=np.float32)
    return dict(at128=at128, b72=b72, r9=r9, g728=g728, h8128=h8128,
                eye=eye, eyer=eye)


def kernel(x, conv_w, bn_gamma, bn_beta, bn_mean, bn_var):
    x = np.ascontiguousarray(np.asarray(x, dtype=np.float32))
    consts = _host_consts(np.asarray(conv_w, np.float32),
                          np.asarray(bn_gamma, np.float32),
                          np.asarray(bn_beta, np.float32),
                          np.asarray(bn_mean, np.float32),
                          np.asarray(bn_var, np.float32))
    nc = _get_program()
    in_maps = [dict(x=x[i], **consts) for i in range(N)]
    res = run_bass_kernel_spmd(nc, in_maps, list(range(N))).results
    low = np.stack([res[i]["low"] for i in range(N)])
    high = np.stack([res[i]["high"] for i in range(N)])
    return low, high


if __name__ == "__main__":
    rng = np.random.default_rng(0)
    demo = dict(
        x=rng.standard_normal((N, IC, H, W), dtype=np.float32),
        conv_w=rng.standard_normal((72, 64)).astype(np.float32),
        bn_gamma=np.ones(72, np.float32),
        bn_beta=np.zeros(72, np.float32),
        bn_mean=rng.standard_normal(72).astype(np.float32) * 0.1,
        bn_var=rng.uniform(0.5, 1.5, 72).astype(np.float32),
    )
    low, high = kernel(**demo)
    print("ok", low.shape, high.shape)


# revision 31
# speedup vs baseline: 1.0083x; 1.0083x over previous
"""Trainium2 Bass kernel for dynamic low-pass filter decomposition.

Module: global-avg-pool -> 1x1 conv -> BN -> softmax over 3x3 taps gives a
per-(sample, group) 3x3 kernel; applied as a reflect-padded depthwise conv
over x; returns (low, x - low).

Sharding: data-parallel over batch n=8 across 8 NeuronCores (1 sample/core).

Per-core layout: partition p = c*2 + h (h = row-half of the image, c =
channel).  Each partition holds 98 rows x 192 cols of its (channel, half)
with one halo row above/below (reflection resolved at DMA time by source row
choice) plus a 1-element front/back pad so tap-shifted views stay in bounds.

Engine split: the TensorEngine accumulates 8 of the 9 taps as diagonal fp32r
matmuls per 512-col PSUM chunk; ScalarE copies the partial out of PSUM;
VectorE adds the 9th (center) tap in-place in SBUF, fixes the reflect
columns at w=0/191 (6 merged ops) and computes high = x - low.  Consts load
on the GpSimd DMA queue so they never queue behind input super-tiles;
warm-up matmuls keep the PE p-state ramped through the input phase; the
1x1-conv/BN/softmax weight chain runs from exact per-chunk partial sums
(fp32) with BN folded into the conv weights on the host.
"""
import sys
import os

sys.path.insert(0, "/opt/trn_rl_repo")

import numpy as np
from contextlib import ExitStack

import concourse.bass as bass
import concourse.tile as tile
from concourse import bacc, mybir
from concourse.bass_utils import run_bass_kernel_spmd

dt = mybir.dt
f32 = dt.float32

KS = 3
GROUP = 8
IC = 64
BN_EPS = 1e-5
N = 8
H = W = 192
RH = 96                 # rows per half-image
NB = 98 * W             # buffer elems per partition (98 rows of 192)
PAD = 1                 # front pad elems (also 1 at the back)
CH = 512                # matmul chunk (one PSUM bank)
# input load chunks (offset, size) in image-region elems, alternating
# between the sync and scalar DMA queues with exactly balanced bytes per
# queue; sizes shrink toward the end so the last partial-sum lands early
LOAD_CHUNKS = [(0, 3072), (3072, 3072), (6144, 3072), (9216, 3072),
               (12288, 2304), (14592, 2304), (16896, 768), (17664, 768)]
SYNC_LOADS = (0, 2, 4, 6)      # chunk idx -> sync queue, rest on scalar
WARM_PER_CHUNK = (2, 2, 2, 2, 2, 1, 1, 1)
WARM_BRIDGE = 4
WARM_CHAIN = 3                 # warm matmuls slotted between chain matmuls
ST_ROWS = [16, 16, 16, 16, 16, 8, 8]      # compute super-tile heights
PE_ALL9 = (6,)                 # drain ST: PE does all 9 taps (short V tail)
PE_TAPS = (0, 1, 2, 3, 5, 6, 7, 8)
V_TAP = 4                      # center tap, added in-place in SBUF


def _build_program():
    """Trace the SPMD Bass program (same for every core)."""
    nc = bacc.Bacc("TRN2", target_bir_lowering=False, debug=False,
                   num_devices=N)

    x_d = nc.dram_tensor("x", [64, H, W], dt.float32r, kind="ExternalInput")
    at_d = nc.dram_tensor("at128", [128, 72], f32, kind="ExternalInput")
    b_d = nc.dram_tensor("b72", [72, 1], f32, kind="ExternalInput")
    r9_d = nc.dram_tensor("r9", [72, 9], f32, kind="ExternalInput")
    g_d = nc.dram_tensor("g728", [72, 8], f32, kind="ExternalInput")
    h_d = nc.dram_tensor("h8128", [8, 128], f32, kind="ExternalInput")
    eye_d = nc.dram_tensor("eye", [128, 128], f32, kind="ExternalInput")
    eyer_d = nc.dram_tensor("eyer", [128, 128], dt.float32r,
                            kind="ExternalInput")
    low_d = nc.dram_tensor("low", [64, H, W], f32, kind="ExternalOutput")
    high_d = nc.dram_tensor("high", [64, H, W], f32, kind="ExternalOutput")

    xt_dram = x_d.ap()

    def dram_flat(tensor, base, inner):
        """Flat (128, inner) AP over DRAM: partition p = c*2 + h covers
        x.flat[p*18432 + base : ... + inner].  Flat leading-dim-128 APs get
        the full 16-engine DMA spray (~430 GB/s); (h,c)-interleaved ones
        only engage 2 engines (~53 GB/s measured)."""
        return bass.AP(tensor, base, [[RH * W, 128], [1, inner]])

    with tile.TileContext(nc) as tc, ExitStack() as ctx:
        cpool = ctx.enter_context(tc.tile_pool(name="static", bufs=1))
        xpool = wpool = cpool
        spool = ctx.enter_context(tc.tile_pool(name="stage", bufs=3))

        # ---- x chunk loads FIRST (queue FIFO position = landing time) on
        # sync+scalar queues; halos behind them on sync; consts go on the
        # GpSimd queue so they land early without delaying x ----
        xt = xpool.tile([128, PAD + NB + 1], dt.float32r)
        # the 1-elem front/back pads are read (never used) by tap-shifted
        # edge views; zero them so they hold no junk/NaN
        nc.vector.memset(xt[:, 0:PAD].bitcast(f32), 0.0)
        nc.vector.memset(xt[:, PAD + NB:PAD + NB + 1].bitcast(f32), 0.0)
        for i, (off, sz) in enumerate(LOAD_CHUNKS):
            eng = nc.sync if i in SYNC_LOADS else nc.scalar
            eng.dma_start(xt[:, PAD + W + off:PAD + W + off + sz],
                          dram_flat(xt_dram.tensor, off, sz))
        # halo row 97 <- image rows {96, 190 (reflect)}[h]; only needed by
        # the last super-tile, so it queues behind sync's chunks
        nc.sync.dma_start(xt[:, PAD + 97 * W:PAD + 98 * W],
                          bass.AP(xt_dram.tensor, 96 * W,
                                  [[H * W, 64], [94 * W, 2], [1, W]]))

        at_s = cpool.tile([128, 72], f32)
        b_s = cpool.tile([72, 1], f32)
        r9_s = cpool.tile([72, 9], f32)
        g_s = cpool.tile([72, 8], f32)
        h_s = cpool.tile([8, 128], f32)
        eye_s = cpool.tile([128, 128], f32)
        eyer_s = cpool.tile([128, 128], dt.float32r)
        for t, d in ((eyer_s, eyer_d), (at_s, at_d), (b_s, b_d),
                     (r9_s, r9_d), (g_s, g_d), (h_s, h_d), (eye_s, eye_d)):
            nc.gpsimd.dma_start(t[:], d.ap())
        # halo row 0 <- image rows {1 (reflect), 95}[h] (8th gp issue: the gp
        # queue has 8 descriptor slots)
        nc.gpsimd.dma_start(xt[:, PAD:PAD + W],
                            bass.AP(xt_dram.tensor, W,
                                    [[H * W, 64], [94 * W, 2], [1, W]]))

        # ---- partial sums per chunk (VectorE reduce / ScalarE accumulate)
        # into one [128, 7] tile; engines split so both overlap the DMAs ----
        pt = wpool.tile([128, 8], f32)
        rscratch = wpool.tile([128, 3072], f32)
        edummy = wpool.tile([72, 1], f32)
        vcol = scol = 0
        for i, (off, sz) in enumerate(LOAD_CHUNKS):
            src = xt[:, PAD + W + off:PAD + W + off + sz].bitcast(f32)
            if i in SYNC_LOADS:
                nc.vector.tensor_reduce(pt[:, vcol:vcol + 1], src,
                                        axis=mybir.AxisListType.X,
                                        op=mybir.AluOpType.add)
                vcol += 1
            else:
                nc.scalar.activation(rscratch[:, 0:sz], src,
                                     mybir.ActivationFunctionType.Copy,
                                     accum_out=pt[:, 4 + scol:5 + scol])
                scol += 1
                if scol == 1:
                    # dummy Exp so the activation table holding Exp is
                    # loaded during the input phase, not on the chain's
                    # critical path
                    nc.scalar.activation(edummy[:], b_s[:],
                                         mybir.ActivationFunctionType.Exp)

        # ---- weight generation chain, sharing one PSUM pool with the PE
        # warm-up matmuls (fewer pools = fewer teardown drain rounds) ----
        sum128 = wpool.tile([128, 1], f32)
        w128 = wpool.tile([128, 9], f32)
        wc = wpool.tile([128, 3], f32)
        diag = [wpool.tile([128, 128], dt.float32r, name=f"diag{k}")
                for k in range(9)]
        wa = PAD + W

        with tc.tile_pool(name="wpsum", bufs=1,
                          space=bass.MemorySpace.PSUM) as wpsum:
            # PE warm-up: dummy matmuls chained to each chunk's landing keep
            # the p-state ramped through the otherwise PE-idle input phase
            wrm2 = wpsum.tile([128, CH], f32, tag="wrm2")
            for i, (off, sz) in enumerate(LOAD_CHUNKS):
                a = PAD + W + off
                for j in range(WARM_PER_CHUNK[i]):
                    nc.tensor.matmul(wrm2[:], eyer_s[:], xt[:, a:a + CH])
            a = PAD + W + LOAD_CHUNKS[-1][0]
            for j in range(WARM_BRIDGE):
                nc.tensor.matmul(wrm2[:], eyer_s[:], xt[:, a:a + CH])

            def warm_chain():
                for j in range(WARM_CHAIN):
                    nc.tensor.matmul(wrm2[:], eyer_s[:], xt[:, wa:wa + CH])

            nc.vector.tensor_reduce(sum128[:], pt[:, 0:8],
                                    axis=mybir.AxisListType.X,
                                    op=mybir.AluOpType.add)
            lf_p = wpsum.tile([72, 1], f32, tag="lf")
            nc.tensor.matmul(lf_p[:], at_s[:], sum128[:])
            warm_chain()
            e72 = wpool.tile([72, 1], f32)
            nc.scalar.activation(e72[:], lf_p[:],
                                 mybir.ActivationFunctionType.Exp,
                                 bias=b_s[:, 0:1], scale=1.0)
            rhsw = wpool.tile([72, 9], f32)
            nc.vector.tensor_scalar_mul(rhsw[:], r9_s[:], e72[:, 0:1])
            w89_p = wpsum.tile([8, 9], f32, tag="w89")
            nc.tensor.matmul(w89_p[:], g_s[:], rhsw[:])
            warm_chain()
            s8 = wpool.tile([8, 1], f32)
            nc.vector.tensor_reduce(s8[:], w89_p[:],
                                    axis=mybir.AxisListType.X,
                                    op=mybir.AluOpType.add)
            r8 = wpool.tile([8, 1], f32)
            nc.vector.reciprocal(r8[:], s8[:])
            w89s = wpool.tile([8, 9], f32)
            nc.vector.tensor_scalar_mul(w89s[:], w89_p[:], r8[:, 0:1])
            wbig_p = wpsum.tile([128, 9], f32, tag="wbig")
            nc.tensor.matmul(wbig_p[:], h_s[:], w89s[:])
            warm_chain()
            # w128 to SBUF; everything downstream (diags, V tap, edge fixes)
            # reads SBUF so this pool closes without gating the main loop
            nc.scalar.copy(w128[:], wbig_p[:])
        # diagonal weight matrices, in ST0's tap order so the PE never waits
        # on a later diag
        for k in PE_TAPS:
            nc.vector.tensor_scalar_mul(diag[k][:], eye_s[:],
                                        w128[:, k:k + 1])
        nc.vector.tensor_scalar_mul(diag[V_TAP][:], eye_s[:],
                                    w128[:, V_TAP:V_TAP + 1])
        # wc merges the dj=0/dj=2 weights hitting the mirror neighbour of a
        # reflected edge column
        nc.vector.tensor_tensor(wc[:], w128[:, 0:9:3], w128[:, 2:9:3],
                                op=mybir.AluOpType.add)

        # ---- main loop ----
        with tc.tile_pool(name="psum", bufs=8,
                          space=bass.MemorySpace.PSUM) as psum:
            r0 = 0
            for s, rows in enumerate(ST_ROWS):
                stw = rows * W
                base = PAD + W + r0 * W
                chunks = []
                o = 0
                while o < stw:
                    chunks.append((o, min(CH, stw - o)))
                    o += CH

                def tap_view(k, lo, sz):
                    di, dj = k // 3, k % 3
                    off = base + lo + (di - 1) * W + (dj - 1)
                    return xt[:, off:off + sz]

                acc = [psum.tile([128, csz], f32, tag="acc",
                                 name=f"acc{s}_{i}")
                       for i, (co, csz) in enumerate(chunks)]
                all9 = s in PE_ALL9
                pe_taps = tuple(range(9)) if all9 else PE_TAPS
                taps = pe_taps if s % 2 == 0 else pe_taps[::-1]
                for k in taps:
                    for i, (co, csz) in enumerate(chunks):
                        nc.tensor.matmul(acc[i][:], diag[k][:],
                                         tap_view(k, co, csz),
                                         start=(k == taps[0]),
                                         stop=(k == taps[-1]))
                low_st = spool.tile([128, stw], f32, tag="low",
                                    padded_shape=[128, 3072])
                # ScalarE drains PSUM; VectorE adds the center tap in-place
                # in SBUF right behind it, chunk by chunk (drain STs do all
                # 9 taps on the PE so the final V tail stays short)
                for i, (co, csz) in enumerate(chunks):
                    dst = low_st[:, co:co + csz]
                    nc.scalar.copy(dst, acc[i][:])
                    if not all9:
                        nc.vector.scalar_tensor_tensor(
                            dst, tap_view(V_TAP, co, csz).bitcast(f32),
                            w128[:, V_TAP:V_TAP + 1], dst,
                            op0=mybir.AluOpType.mult,
                            op1=mybir.AluOpType.add)
                # edge-column fixes (reflect at w=0 and w=191): per di, the
                # edge output is w[di,1]*x[.,edge] + (w[di,0]+w[di,2])*x[.,
                # mirror-neighbour]; both columns per op via strided views
                out_ap = low_st[:, 0:stw].rearrange(
                    "p (r w) -> p r w", w=W)[:, :, 0:W:W - 1]
                for di in range(3):
                    vb = PAD + (r0 + di) * W
                    va = xt[:, vb:vb + stw].bitcast(f32).rearrange(
                        "p (r w) -> p r w", w=W)[:, :, 0:W:W - 1]
                    vn = xt[:, vb + 1:vb + 1 + stw].bitcast(f32).rearrange(
                        "p (r w) -> p r w", w=W)[:, :, 0:190:189]
                    if di == 0:
                        nc.vector.tensor_scalar_mul(out_ap, va, w128[:, 1:2])
                    else:
                        nc.vector.scalar_tensor_tensor(
                            out_ap, va, w128[:, 3 * di + 1:3 * di + 2],
                            out_ap, op0=mybir.AluOpType.mult,
                            op1=mybir.AluOpType.add)
                    nc.vector.scalar_tensor_tensor(
                        out_ap, vn, wc[:, di:di + 1], out_ap,
                        op0=mybir.AluOpType.mult,
                        op1=mybir.AluOpType.add)
                high_st = spool.tile([128, stw], f32, tag="high",
                                     padded_shape=[128, 3072])
                nc.vector.tensor_tensor(high_st[:],
                                        xt[:, base:base + stw].bitcast(f32),
                                        low_st[:],
                                        op=mybir.AluOpType.subtract)
                nc.scalar.dma_start(
                    dram_flat(low_d.ap().tensor, r0 * W, stw), low_st[:])
                nc.sync.dma_start(
                    dram_flat(high_d.ap().tensor, r0 * W, stw), high_st[:])
                r0 += rows

    nc.compile()
    return nc


def _enable_ldw_opt():
    """walrus emits one LDWEIGHTS per matmul with --enable-ldw-opt=false
    (most are redundant reloads of the same diagonal).  Rewrite the flag on
    the compiler command line."""
    import concourse.bass_utils as BU
    if getattr(BU, "_ldw_patched", False):
        return
    orig = BU.run_command

    def patched(cmd, *a, **kw):
        cmd = [c.replace("--enable-ldw-opt=false", "--enable-ldw-opt=true")
               if isinstance(c, str) else c for c in cmd]
        return orig(cmd, *a, **kw)

    BU.run_command = patched
    BU._ldw_patched = True


_nc_cache = None


def _get_program():
    global _nc_cache
    if _nc_cache is None:
        _enable_ldw_opt()
        _nc_cache = _build_program()
    return _nc_cache


def _host_consts(conv_w, bn_gamma, bn_beta, bn_mean, bn_var):
    s_a = bn_gamma / np.sqrt(bn_var + BN_EPS)
    b72 = (bn_beta - bn_mean * s_a).astype(np.float32).reshape(72, 1)
    A = (conv_w * s_a[:, None]) / np.float32(H * W)
    p = np.arange(128)
    at128 = np.ascontiguousarray(A.T[p // 2]).astype(np.float32)  # (128, 72)
    oc = np.arange(72)
    r9 = (oc[:, None] % 9 == np.arange(9)[None, :]).astype(np.float32)
    g728 = (oc[:, None] // 9 == np.arange(8)[None, :]).astype(np.float32)
    h8128 = (np.arange(8)[:, None] == (p[None, :] // 16)).astype(np.float32)
    eye = np.eye(128, dtype=np.float32)
    return dict(at128=at128, b72=b72, r9=r9, g728=g728, h8128=h8128,
                eye=eye, eyer=eye)


def kernel(x, conv_w, bn_gamma, bn_beta, bn_mean, bn_var):
    x = np.ascontiguousarray(np.asarray(x, dtype=np.float32))
    consts = _host_consts(np.asarray(conv_w, np.float32),
                          np.asarray(bn_gamma, np.float32),
                          np.asarray(bn_beta, np.float32),
                          np.asarray(bn_mean, np.float32),
                          np.asarray(bn_var, np.float32))
    nc = _get_program()
    in_maps = [dict(x=x[i], **consts) for i in range(N)]
    res = run_bass_kernel_spmd(nc, in_maps, list(range(N))).results
    low = np.stack([res[i]["low"] for i in range(N)])
    high = np.stack([res[i]["high"] for i in range(N)])
    return low, high


if __name__ == "__main__":
    rng = np.random.default_rng(0)
    demo = dict(
        x=rng.standard_normal((N, IC, H, W), dtype=np.float32),
        conv_w=rng.standard_normal((72, 64)).astype(np.float32),
        bn_gamma=np.ones(72, np.float32),
        bn_beta=np.zeros(72, np.float32),
        bn_mean=rng.standard_normal(72).astype(np.float32) * 0.1,
        bn_var=rng.uniform(0.5, 1.5, 72).astype(np.float32),
    )
    low, high = kernel(**demo)
    print("ok", low.shape, high.shape)
